# revision 17
# baseline (speedup 1.0000x reference)
"""8x8 block DCT (DCT-II) on [64,1,1024,1024] fp32 -> [64,64,128,128].

Data parallel over batch: 8 images per NeuronCore on 8 cores.

Per 128x128 image tile T, the 2D DCT of all 256 8x8 blocks is two dense
PE matmuls against one constant block-diagonal permuted DCT matrix DT1
(DT1[8*b + x, 16*u + b] = M[u, x]):
    U = T^T @ DT1        [c, 16u+bi]     (stage 1, fp32)
    Z = U^T @ DT1        [16u+bi, 16v+bj] (stage 2, fp16 hi/lo x3, ~1e-6 rel)
Stage 2 splits U into fp16 hi+lo during the mandatory PSUM drain and uses
fp16 hi/lo DCT constants, accumulating three fp16 matmuls in PSUM: full
fp32-grade accuracy at 1 cycle/row instead of 4.

Z is scatter-drained into a per-image SBUF buffer laid out [p=16u+bi,
f = v*1024 + ti*128 + J] so each (img, u) stores with ONE 512KB DMA whose
3-dim AP covers 8 output channels. Output descriptors are 512B (forced:
block-row index bi lives on partitions); throughput recovers by spreading
descriptor generation across the three DGE paths (SP-HWDGE, ACT-HWDGE,
GPSIMD-SWDGE).
"""

import numpy as np

_N_CORES = 8
_H = 1024
_W = 1024

_NC_CACHE = {}

# tuning knobs
OUT_ENGINES = "sscg"  # cycle pattern: s=sync, c=scalar, g=gpsimd
IN_ENGINE = "g"
GROUP = 4  # tiles per PSUM bank group (must divide 8)
SCATTER_SPLIT = True
ZIMG_BUFS = 3
XS_BUFS = 3
HOST_SPLIT = False
S2_SINGLE = True  # stage 2: one fp16 matmul instead of hi/lo x3
S1_F32R = False  # rejected by walrus: no mixing 32-bit/16-bit matmul inputs


def _dct_mat_np():
    n = 8
    u = np.arange(n)[:, None].astype(np.float64)
    x = np.arange(n)[None, :].astype(np.float64)
    m = np.cos((2 * x + 1) * u * np.pi / (2 * n))
    scale = np.where(u == 0, np.sqrt(1.0 / n), np.sqrt(2.0 / n))
    return (m * scale).astype(np.float32)


def _build_dt1(dct: np.ndarray) -> np.ndarray:
    """DT1[8*b + x, 16*u + b] = dct[u, x], zero elsewhere."""
    dt1 = np.zeros((128, 128), dtype=np.float32)
    for b in range(16):
        dt1[8 * b : 8 * b + 8, b::16] = dct.T
    return dt1


def build_nc(
    n_img: int,
    out_engines=OUT_ENGINES,
    in_engine=IN_ENGINE,
    group=GROUP,
    scatter_split=SCATTER_SPLIT,
    zimg_bufs=ZIMG_BUFS,
    xs_bufs=XS_BUFS,
    strip_input=False,
    host_split=HOST_SPLIT,
    s2_single=S2_SINGLE,
    s1_f32r=S1_F32R,
):
    import concourse.bacc as bacc
    import concourse.mybir as mybir
    import concourse.tile as tile

    f32 = mybir.dt.float32
    f32r = mybir.dt.float32r
    f16 = mybir.dt.float16
    nc = bacc.Bacc("TRN2", target_bir_lowering=False, debug=False)

    if host_split:
        x = nc.dram_tensor("x", [n_img, 1, _H, 2 * _W], f16, kind="ExternalInput")
    else:
        x = nc.dram_tensor("x", [n_img, 1, _H, _W], f32, kind="ExternalInput")
    dt1 = nc.dram_tensor("dt1", [128, 128], f32, kind="ExternalInput")
    dt1h = nc.dram_tensor("dt1h", [128, 128], f16, kind="ExternalInput")
    dt1l = nc.dram_tensor("dt1l", [128, 128], f16, kind="ExternalInput")
    out = nc.dram_tensor("out", [n_img, 64, 128, 128], f32, kind="ExternalOutput")

    def eng(ch):
        return {"s": nc.sync, "c": nc.scalar, "g": nc.gpsimd}[ch]

    n_out_dma = 0

    with tile.TileContext(nc) as tc:
        with (
            tc.tile_pool(name="const", bufs=1) as constp,
            tc.tile_pool(
                name="xs", bufs=(xs_bufs * 8 if strip_input else xs_bufs)
            ) as xsp,
            tc.tile_pool(name="zimg", bufs=zimg_bufs) as zp,
            tc.tile_pool(name="uhi", bufs=3) as uhip,
            tc.tile_pool(name="ulo", bufs=3) as ulop,
            tc.tile_pool(name="psu", bufs=(3 if group <= 4 else 2), space="PSUM") as psu,
            tc.tile_pool(name="psz", bufs=(3 if group <= 4 else 2), space="PSUM") as psz,
        ):
            dt1_t = constp.tile([128, 128], f32)
            nc.sync.dma_start(dt1_t[:], dt1[:])
            dt1h_t = constp.tile([128, 128], f16)
            nc.sync.dma_start(dt1h_t[:], dt1h[:])
            dt1l_t = constp.tile([128, 128], f16)
            nc.sync.dma_start(dt1l_t[:], dt1l[:])

            for img in range(n_img):
                if host_split:
                    # xs[p, s*2048 + c] = x[img, 0, 128*s+p, c]; row = hi|lo
                    xs = xsp.tile([128, 8 * 2 * _W], f16)
                    src = x[img, 0, :, :].rearrange("(s p) c -> p s c", p=128)
                    eng(in_engine).dma_start(
                        xs[:].rearrange("p (s c) -> p s c", s=8), src
                    )
                elif strip_input:
                    xstrips = []
                    for ti in range(8):
                        xst = xsp.tile([128, _W], f32, tag="xstrip")
                        eng(in_engine).dma_start(
                            xst[:], x[img, 0, 128 * ti : 128 * (ti + 1), :]
                        )
                        xstrips.append(xst)
                else:
                    # Load full image: xs[p, s*1024 + c] = x[img, 0, 128*s+p, c]
                    xs = xsp.tile([128, 8 * _W], f32)
                    src = x[img, 0, :, :].rearrange("(s p) c -> p s c", p=128)
                    eng(in_engine).dma_start(
                        xs[:].rearrange("p (s c) -> p s c", s=8), src
                    )

                # Zimg[p=16u+bi, v*1024 + ti*128 + tj*16 + bj]
                zimg = zp.tile([128, 8 * _W], f32)

                for ti in range(8):
                    for tj0 in range(0, 8, group):
                        gw = group * 128
                        u_ps = psu.tile([128, gw], f32)
                        for q in range(group):
                            tj = tj0 + q
                            uq = u_ps[:, q * 128 : (q + 1) * 128]
                            if host_split:
                                hi = xs[
                                    :,
                                    ti * 2048 + tj * 128 : ti * 2048 + (tj + 1) * 128,
                                ]
                                lo = xs[
                                    :,
                                    ti * 2048 + 1024 + tj * 128 : ti * 2048
                                    + 1024
                                    + (tj + 1) * 128,
                                ]
                                nc.tensor.matmul(
                                    uq, hi, dt1h_t[:], start=True, stop=False
                                )
                                nc.tensor.matmul(
                                    uq, hi, dt1l_t[:], start=False, stop=False
                                )
                                nc.tensor.matmul(
                                    uq, lo, dt1h_t[:], start=False, stop=True
                                )
                                continue
                            if strip_input:
                                lhs = xstrips[ti][:, tj * 128 : (tj + 1) * 128]
                            else:
                                lhs = xs[
                                    :,
                                    ti * 1024 + tj * 128 : ti * 1024 + (tj + 1) * 128,
                                ]
                            if s1_f32r:
                                nc.tensor.matmul(
                                    uq,
                                    lhs.bitcast(f32r),
                                    dt1h_t[:],
                                    start=True,
                                    stop=True,
                                )
                            else:
                                nc.tensor.matmul(
                                    uq,
                                    lhs,
                                    dt1_t[:],
                                    start=True,
                                    stop=True,
                                )
                        u_hi = uhip.tile([128, gw], f16)
                        nc.scalar.copy(u_hi[:], u_ps[:])
                        if not s2_single:
                            u_lo = ulop.tile([128, gw], f16)
                            nc.vector.tensor_sub(u_lo[:], u_ps[:], u_hi[:])

                        z_ps = psz.tile([128, gw], f32)
                        for q in range(group):
                            zq = z_ps[:, q * 128 : (q + 1) * 128]
                            hi_q = u_hi[:, q * 128 : (q + 1) * 128]
                            if s2_single:
                                nc.tensor.matmul(
                                    zq, hi_q, dt1h_t[:], start=True, stop=True
                                )
                                continue
                            lo_q = u_lo[:, q * 128 : (q + 1) * 128]
                            nc.tensor.matmul(
                                zq, hi_q, dt1h_t[:], start=True, stop=False
                            )
                            nc.tensor.matmul(
                                zq, hi_q, dt1l_t[:], start=False, stop=False
                            )
                            nc.tensor.matmul(
                                zq, lo_q, dt1h_t[:], start=False, stop=True
                            )

                        # scatter: z_ps[p, q*128 + 16v + bj]
                        #   -> zimg[p, v*1024 + ti*128 + (tj0+q)*16 + bj]
                        src4 = z_ps[:].rearrange("p (q v b) -> p q v b", q=group, v=8)
                        dstv = zimg[:].rearrange(
                            "p (v t j) -> p v t j", v=8, t=8
                        )[:, :, ti, tj0 * 16 : tj0 * 16 + group * 16]
                        dst4 = dstv.rearrange("p v (q b) -> p q v b", q=group)
                        if scatter_split and (ti * (8 // group) + tj0 // group) % 2:
                            nc.scalar.copy(dst4, src4)
                        else:
                            nc.vector.tensor_copy(dst4, src4)

                # Store: one fat DMA per u covering channels 8u..8u+8
                for u in range(8):
                    src = zimg[16 * u : 16 * u + 16, :]
                    dst = out[img, 8 * u : 8 * u + 8, :, :].rearrange(
                        "v (t b) j -> b (v t) j", b=16
                    )
                    e = out_engines[n_out_dma % len(out_engines)]
                    n_out_dma += 1
                    eng(e).dma_start(dst, src)

    nc.compile()
    return nc


def build_nc_v3(
    n_img: int,
    in_engine="s",
    out_engines="ssgc",
    u2_engines="vc",
    s0_engines="vc",
    utd_engine="v",
    t1d_engine="c",
    t2d_engines="cv",
    xst_bufs=6,
    u2sb_bufs=3,
    utsb_bufs=3,
    s0_bufs=2,
    s1_bufs=2,
    s2_bufs=2,
    ptp_bufs=4,
):
    """Channel-major output, f32r stage 1 (no input conversion).

    Per image (8 strips x 8 tiles of 128x128):
      S1 (PE, f32r x f32r, 1 cyc/row at 512-wide): per half-strip
         u2[16u+bi, c] = DT1.T @ xstrip            (PSUM f32)
      u2 drain -> u2sb fp16 [128, 1024]            (cast, contiguous)
      UT (PE): transpose 128-chunks -> UTsb[c_loc, tj*?? + 16u+bi] fp16
      S2 (PE, fp16): z2[16v+bj, (q,u,bi)] = DT1h.T @ UTsb_half
      S0[p0=16v+bj, ti*1024 + u*128 + tj*16 + bi]  (cast, 16-elem runs)
      T1 (PE): transpose S0[:, ti*1024+u*128 :+128] -> P1[tj*16+bi, 16v+bj]
      S1sb[p1, ti2*4096 + bj*256 + til*64 + u*8 + v]  (fp16, 8-elem runs)
      T2 (PE, half-width): S1sb 64-windows @ (ti2,bj,til)
         -> P2[ti2*64 + u*8 + v, tj*16+bi]
      S2sb[p2, til*2048 + bi*128 + tj*16 + bj]     (fp32, 4-elem runs)
      out DMA per (u, ti2): 8 partitions x 32KB contiguous descriptors.
    """
    import concourse.bacc as bacc
    import concourse.mybir as mybir
    import concourse.tile as tile

    f32 = mybir.dt.float32
    f32r = mybir.dt.float32r
    f16 = mybir.dt.float16
    nc = bacc.Bacc("TRN2", target_bir_lowering=False, debug=False)

    x = nc.dram_tensor("x", [n_img, 1, _H, _W], f32r, kind="ExternalInput")
    dt1 = nc.dram_tensor("dt1", [128, 128], f32r, kind="ExternalInput")
    dt1h = nc.dram_tensor("dt1h", [128, 128], f16, kind="ExternalInput")
    ident = nc.dram_tensor("ident", [128, 128], f16, kind="ExternalInput")
    out = nc.dram_tensor("out", [n_img, 64, 128, 128], f32, kind="ExternalOutput")

    def eng(ch):
        return {"s": nc.sync, "c": nc.scalar, "g": nc.gpsimd, "v": nc.vector}[ch]

    def ecopy(ch, out_ap, in_ap):
        if ch == "c":
            nc.scalar.copy(out_ap, in_ap)
        else:
            eng(ch).tensor_copy(out_ap, in_ap)

    cnt = {"u2": 0, "s0": 0, "t2": 0, "out": 0}

    with tile.TileContext(nc) as tc:
        with (
            tc.tile_pool(name="const", bufs=1) as constp,
            tc.tile_pool(name="xst", bufs=xst_bufs) as xsp,
            tc.tile_pool(name="u2sb", bufs=u2sb_bufs) as u2p,
            tc.tile_pool(name="utsb", bufs=utsb_bufs) as utp,
            tc.tile_pool(name="s0", bufs=s0_bufs) as s0p,
            tc.tile_pool(name="s1sb", bufs=s1_bufs) as s1p,
            tc.tile_pool(name="s2sb", bufs=s2_bufs) as s2p,
            tc.tile_pool(name="psu", bufs=2, space="PSUM") as psu,
            tc.tile_pool(name="psz", bufs=2, space="PSUM") as psz,
            tc.tile_pool(name="ptp", bufs=ptp_bufs, space="PSUM") as ptp,
        ):
            dt1_t = constp.tile([128, 128], f32r)
            nc.sync.dma_start(dt1_t[:], dt1[:])
            dt1h_t = constp.tile([128, 128], f16)
            nc.sync.dma_start(dt1h_t[:], dt1h[:])
            ident_t = constp.tile([128, 128], f16)
            nc.sync.dma_start(ident_t[:], ident[:])

            for img in range(n_img):
                s0 = s0p.tile([128, 8192], f16)
                for ti in range(8):
                    xst = xsp.tile([128, _W], f32r, tag="xstrip")
                    eng(in_engine).dma_start(
                        xst[:], x[img, 0, 128 * ti : 128 * (ti + 1), :]
                    )
                    # stage 1 (f32r): u2[16u+bi, c] per half-strip
                    u2sb = u2p.tile([128, 1024], f16)
                    for h in range(2):
                        u2_ps = psu.tile([128, 512], f32)
                        nc.tensor.matmul(
                            u2_ps[:],
                            dt1_t[:],
                            xst[:, h * 512 : (h + 1) * 512],
                            start=True,
                            stop=True,
                        )
                        e = u2_engines[cnt["u2"] % len(u2_engines)]
                        cnt["u2"] += 1
                        ecopy(e, u2sb[:, h * 512 : (h + 1) * 512], u2_ps[:])
                    # UT: back to [c_loc, 16u+bi] per 128-chunk
                    put = ptp.tile([128, 1024], f16, tag="tp")
                    for cc in range(8):
                        nc.tensor.transpose(
                            put[:, cc * 128 : (cc + 1) * 128],
                            u2sb[:, cc * 128 : (cc + 1) * 128],
                            ident_t[:],
                        )
                    utsb = utp.tile([128, 1024], f16)
                    ecopy(utd_engine, utsb[:], put[:])
                    # stage 2 (fp16) + S0 drain
                    for h in range(2):
                        z2 = psz.tile([128, 512], f32)
                        nc.tensor.matmul(
                            z2[:],
                            dt1h_t[:],
                            utsb[:, h * 512 : (h + 1) * 512],
                            start=True,
                            stop=True,
                        )
                        src = z2[:].rearrange("p (q u b) -> p q u b", q=4, u=8)
                        dst = s0[:].rearrange(
                            "p (t u j b) -> p t j u b", t=8, u=8, j=8
                        )[:, ti, h * 4 : h * 4 + 4, :, :]
                        e = s0_engines[cnt["s0"] % len(s0_engines)]
                        cnt["s0"] += 1
                        ecopy(e, dst, src)

                # ---- T1: per ti, 8 transposes (u) into one PSUM bank ----
                s1sb = s1p.tile([128, 8192], f16)
                s1v = s1sb[:].rearrange(
                    "p (w b t u v) -> p w t u v b", w=2, b=16, t=4, u=8
                )
                for ti in range(8):
                    ti2, til = ti // 4, ti % 4
                    pt = ptp.tile([128, 1024], f16, tag="tp")
                    for u in range(8):
                        nc.tensor.transpose(
                            pt[:, u * 128 : (u + 1) * 128],
                            s0[:, ti * 1024 + u * 128 : ti * 1024 + (u + 1) * 128],
                            ident_t[:],
                        )
                    src = pt[:].rearrange("p (u v b) -> p u v b", u=8, v=8)
                    dst = s1v[:, ti2, til, :, :, :]
                    ecopy(t1d_engine, dst, src)

                # ---- T2: per (til, g2), 16 half-width transposes ----
                s2sb = s2p.tile([128, 8192], f32)
                s2v = s2sb[:].rearrange("p (t b j c) -> p t c j b", t=4, b=16, j=8)
                for til in range(4):
                    for g2 in range(2):
                        pt2 = ptp.tile([128, 1024], f16, tag="tp")
                        for kk in range(8):
                            bj = g2 * 8 + kk
                            for ti2 in range(2):
                                nc.tensor.transpose(
                                    pt2[
                                        ti2 * 64 : ti2 * 64 + 64,
                                        kk * 128 : (kk + 1) * 128,
                                    ],
                                    s1sb[
                                        :,
                                        ti2 * 4096 + bj * 256 + til * 64 : ti2 * 4096
                                        + bj * 256
                                        + til * 64
                                        + 64,
                                    ],
                                    ident_t[:],
                                )
                        src = pt2[:].rearrange("p (c t b) -> p c t b", c=8, t=8)
                        dst = s2v[:, til, g2 * 8 : g2 * 8 + 8, :, :]
                        e = t2d_engines[cnt["t2"] % len(t2d_engines)]
                        cnt["t2"] += 1
                        ecopy(e, dst, src)

                # ---- output: per (u, ti2): 8 partitions x 32KB ----
                for u in range(8):
                    for ti2 in range(2):
                        src = s2sb[ti2 * 64 + u * 8 : ti2 * 64 + (u + 1) * 8, :]
                        dst = out[
                            img, 8 * u : 8 * u + 8, ti2 * 64 : ti2 * 64 + 64, :
                        ].rearrange("v r j -> v (r j)")
                        e = out_engines[cnt["out"] % len(out_engines)]
                        cnt["out"] += 1
                        eng(e).dma_start(dst, src)

    nc.compile()
    return nc


def _get_nc(n_img: int):
    if n_img not in _NC_CACHE:
        _NC_CACHE[n_img] = build_nc_v3(n_img)
    return _NC_CACHE[n_img]


def _split_f16(m: np.ndarray):
    hi = m.astype(np.float16)
    lo = (m - hi.astype(np.float32)).astype(np.float16)
    return hi, lo


def make_inputs(x_core: np.ndarray, dct: np.ndarray, host_split=False, v2=True) -> dict:
    dt1 = _build_dt1(dct)
    dt1h, dt1l = _split_f16(dt1)
    if v2:
        ident = np.eye(128, dtype=np.float16)
        return {"x": x_core, "dt1": dt1, "dt1h": dt1h, "ident": ident}
    if host_split:
        xh = x_core.astype(np.float16)
        xl = (x_core - xh.astype(np.float32)).astype(np.float16)
        x_core = np.concatenate((xh, xl), axis=-1)
    return {"x": x_core, "dt1": dt1, "dt1h": dt1h, "dt1l": dt1l}


def run_spmd(
    x: np.ndarray, dct: np.ndarray, trace: bool = False, nc=None, host_split=HOST_SPLIT
):
    """Run the SPMD kernel on 8 cores. Returns (out, BassKernelResults)."""
    from concourse.bass_utils import run_bass_kernel_spmd

    x = np.ascontiguousarray(np.asarray(x, dtype=np.float32))
    dct = np.asarray(dct, dtype=np.float32)
    b = x.shape[0]
    per = b // _N_CORES

    if nc is None:
        nc = _get_nc(per)
    in_maps = [
        make_inputs(x[i * per : (i + 1) * per], dct, host_split=host_split)
        for i in range(_N_CORES)
    ]
    res = run_bass_kernel_spmd(
        nc, in_maps, core_ids=list(range(_N_CORES)), trace=trace
    )
    out = np.concatenate(
        [res.results[i]["out"] for i in range(_N_CORES)], axis=0
    )
    return out, res


def kernel(x, dct=None):
    if dct is None:
        dct = _dct_mat_np()
    out, _ = run_spmd(x, dct, trace=False)
    return out



# revision 20
# speedup vs baseline: 2.8255x; 2.8255x over previous
"""8x8 block DCT (DCT-II) on [64,1,1024,1024] fp32 -> [64,64,128,128].

Data parallel over batch: 8 images per NeuronCore on 8 cores.

Per 128x128 image tile T, the 2D DCT of all 256 8x8 blocks is two dense
PE matmuls against one constant block-diagonal permuted DCT matrix DT1
(DT1[8*b + x, 16*u + b] = M[u, x]):
    U = T^T @ DT1        [c, 16u+bi]     (stage 1, fp32)
    Z = U^T @ DT1        [16u+bi, 16v+bj] (stage 2, fp16 hi/lo x3, ~1e-6 rel)
Stage 2 splits U into fp16 hi+lo during the mandatory PSUM drain and uses
fp16 hi/lo DCT constants, accumulating three fp16 matmuls in PSUM: full
fp32-grade accuracy at 1 cycle/row instead of 4.

Z is scatter-drained into a per-image SBUF buffer laid out [p=16u+bi,
f = v*1024 + ti*128 + J] so each (img, u) stores with ONE 512KB DMA whose
3-dim AP covers 8 output channels. Output descriptors are 512B (forced:
block-row index bi lives on partitions); throughput recovers by spreading
descriptor generation across the three DGE paths (SP-HWDGE, ACT-HWDGE,
GPSIMD-SWDGE).
"""

import numpy as np

_N_CORES = 8
_H = 1024
_W = 1024

_NC_CACHE = {}

# tuning knobs
OUT_ENGINES = "sscg"  # cycle pattern: s=sync, c=scalar, g=gpsimd
IN_ENGINE = "g"
GROUP = 4  # tiles per PSUM bank group (must divide 8)
SCATTER_SPLIT = True
ZIMG_BUFS = 3
XS_BUFS = 3
HOST_SPLIT = False
S2_SINGLE = True  # stage 2: one fp16 matmul instead of hi/lo x3
S1_F32R = False  # rejected by walrus: no mixing 32-bit/16-bit matmul inputs
STRIP_INPUT = True


def _dct_mat_np():
    n = 8
    u = np.arange(n)[:, None].astype(np.float64)
    x = np.arange(n)[None, :].astype(np.float64)
    m = np.cos((2 * x + 1) * u * np.pi / (2 * n))
    scale = np.where(u == 0, np.sqrt(1.0 / n), np.sqrt(2.0 / n))
    return (m * scale).astype(np.float32)


def _build_dt1(dct: np.ndarray) -> np.ndarray:
    """DT1[8*b + x, 16*u + b] = dct[u, x], zero elsewhere."""
    dt1 = np.zeros((128, 128), dtype=np.float32)
    for b in range(16):
        dt1[8 * b : 8 * b + 8, b::16] = dct.T
    return dt1


def build_nc(
    n_img: int,
    out_engines=OUT_ENGINES,
    in_engine=IN_ENGINE,
    group=GROUP,
    scatter_split=SCATTER_SPLIT,
    zimg_bufs=ZIMG_BUFS,
    xs_bufs=XS_BUFS,
    strip_input=False,
    host_split=HOST_SPLIT,
    s2_single=S2_SINGLE,
    s1_f32r=S1_F32R,
    half_flush=False,
):
    import concourse.bacc as bacc
    import concourse.mybir as mybir
    import concourse.tile as tile

    f32 = mybir.dt.float32
    f32r = mybir.dt.float32r
    f16 = mybir.dt.float16
    nc = bacc.Bacc("TRN2", target_bir_lowering=False, debug=False)

    if host_split:
        x = nc.dram_tensor("x", [n_img, 1, _H, 2 * _W], f16, kind="ExternalInput")
    else:
        x = nc.dram_tensor("x", [n_img, 1, _H, _W], f32, kind="ExternalInput")
    dt1 = nc.dram_tensor("dt1", [128, 128], f32, kind="ExternalInput")
    dt1h = nc.dram_tensor("dt1h", [128, 128], f16, kind="ExternalInput")
    dt1l = nc.dram_tensor("dt1l", [128, 128], f16, kind="ExternalInput")
    out = nc.dram_tensor("out", [n_img, 64, 128, 128], f32, kind="ExternalOutput")

    def eng(ch):
        return {"s": nc.sync, "c": nc.scalar, "g": nc.gpsimd}[ch]

    n_out_dma = 0

    with tile.TileContext(nc) as tc:
        with (
            tc.tile_pool(name="const", bufs=1) as constp,
            tc.tile_pool(
                name="xs", bufs=(xs_bufs * 8 if strip_input else xs_bufs)
            ) as xsp,
            tc.tile_pool(name="zimg", bufs=zimg_bufs) as zp,
            tc.tile_pool(name="uhi", bufs=3) as uhip,
            tc.tile_pool(name="ulo", bufs=3) as ulop,
            tc.tile_pool(name="psu", bufs=(3 if group <= 4 else 2), space="PSUM") as psu,
            tc.tile_pool(name="psz", bufs=(3 if group <= 4 else 2), space="PSUM") as psz,
        ):
            dt1_t = constp.tile([128, 128], f32)
            nc.sync.dma_start(dt1_t[:], dt1[:])
            dt1h_t = constp.tile([128, 128], f16)
            nc.sync.dma_start(dt1h_t[:], dt1h[:])
            dt1l_t = constp.tile([128, 128], f16)
            nc.sync.dma_start(dt1l_t[:], dt1l[:])

            for img in range(n_img):
                if host_split:
                    # xs[p, s*2048 + c] = x[img, 0, 128*s+p, c]; row = hi|lo
                    xs = xsp.tile([128, 8 * 2 * _W], f16)
                    src = x[img, 0, :, :].rearrange("(s p) c -> p s c", p=128)
                    eng(in_engine).dma_start(
                        xs[:].rearrange("p (s c) -> p s c", s=8), src
                    )
                elif strip_input:
                    xstrips = []
                    for ti in range(8):
                        xst = xsp.tile([128, _W], f32, tag="xstrip")
                        eng(in_engine).dma_start(
                            xst[:], x[img, 0, 128 * ti : 128 * (ti + 1), :]
                        )
                        xstrips.append(xst)
                else:
                    # Load full image: xs[p, s*1024 + c] = x[img, 0, 128*s+p, c]
                    xs = xsp.tile([128, 8 * _W], f32)
                    src = x[img, 0, :, :].rearrange("(s p) c -> p s c", p=128)
                    eng(in_engine).dma_start(
                        xs[:].rearrange("p (s c) -> p s c", s=8), src
                    )

                # Zimg[p=16u+bi, v*1024 + ti*128 + tj*16 + bj]
                zimg = zp.tile([128, 8 * _W], f32)

                for ti in range(8):
                    for tj0 in range(0, 8, group):
                        gw = group * 128
                        u_ps = psu.tile([128, gw], f32)
                        for q in range(group):
                            tj = tj0 + q
                            uq = u_ps[:, q * 128 : (q + 1) * 128]
                            if host_split:
                                hi = xs[
                                    :,
                                    ti * 2048 + tj * 128 : ti * 2048 + (tj + 1) * 128,
                                ]
                                lo = xs[
                                    :,
                                    ti * 2048 + 1024 + tj * 128 : ti * 2048
                                    + 1024
                                    + (tj + 1) * 128,
                                ]
                                nc.tensor.matmul(
                                    uq, hi, dt1h_t[:], start=True, stop=False
                                )
                                nc.tensor.matmul(
                                    uq, hi, dt1l_t[:], start=False, stop=False
                                )
                                nc.tensor.matmul(
                                    uq, lo, dt1h_t[:], start=False, stop=True
                                )
                                continue
                            if strip_input:
                                lhs = xstrips[ti][:, tj * 128 : (tj + 1) * 128]
                            else:
                                lhs = xs[
                                    :,
                                    ti * 1024 + tj * 128 : ti * 1024 + (tj + 1) * 128,
                                ]
                            if s1_f32r:
                                nc.tensor.matmul(
                                    uq,
                                    lhs.bitcast(f32r),
                                    dt1h_t[:],
                                    start=True,
                                    stop=True,
                                )
                            else:
                                nc.tensor.matmul(
                                    uq,
                                    lhs,
                                    dt1_t[:],
                                    start=True,
                                    stop=True,
                                )
                        u_hi = uhip.tile([128, gw], f16)
                        nc.scalar.copy(u_hi[:], u_ps[:])
                        if not s2_single:
                            u_lo = ulop.tile([128, gw], f16)
                            nc.vector.tensor_sub(u_lo[:], u_ps[:], u_hi[:])

                        z_ps = psz.tile([128, gw], f32)
                        for q in range(group):
                            zq = z_ps[:, q * 128 : (q + 1) * 128]
                            hi_q = u_hi[:, q * 128 : (q + 1) * 128]
                            if s2_single:
                                nc.tensor.matmul(
                                    zq, hi_q, dt1h_t[:], start=True, stop=True
                                )
                                continue
                            lo_q = u_lo[:, q * 128 : (q + 1) * 128]
                            nc.tensor.matmul(
                                zq, hi_q, dt1h_t[:], start=True, stop=False
                            )
                            nc.tensor.matmul(
                                zq, hi_q, dt1l_t[:], start=False, stop=False
                            )
                            nc.tensor.matmul(
                                zq, lo_q, dt1h_t[:], start=False, stop=True
                            )

                        # scatter: z_ps[p, q*128 + 16v + bj]
                        #   -> zimg[p, v*1024 + ti*128 + (tj0+q)*16 + bj]
                        src4 = z_ps[:].rearrange("p (q v b) -> p q v b", q=group, v=8)
                        dstv = zimg[:].rearrange(
                            "p (v t j) -> p v t j", v=8, t=8
                        )[:, :, ti, tj0 * 16 : tj0 * 16 + group * 16]
                        dst4 = dstv.rearrange("p v (q b) -> p q v b", q=group)
                        if scatter_split and (ti * (8 // group) + tj0 // group) % 2:
                            nc.scalar.copy(dst4, src4)
                        else:
                            nc.vector.tensor_copy(dst4, src4)

                    if half_flush and ti in (3, 7):
                        hh = ti // 4
                        zv = zimg[:].rearrange("p (v t j) -> p v t j", v=8, t=8)
                        for u in range(8):
                            src = zv[16 * u : 16 * u + 16, :, hh * 4 : hh * 4 + 4, :]
                            dst = out[
                                img, 8 * u : 8 * u + 8, hh * 64 : hh * 64 + 64, :
                            ].rearrange("v (t b) j -> b v t j", b=16)
                            e = out_engines[n_out_dma % len(out_engines)]
                            n_out_dma += 1
                            eng(e).dma_start(dst, src)

                # Store: fat DMAs; optionally split per ti-half for earlier flush
                if not half_flush:
                    for u in range(8):
                        src = zimg[16 * u : 16 * u + 16, :]
                        dst = out[img, 8 * u : 8 * u + 8, :, :].rearrange(
                            "v (t b) j -> b (v t) j", b=16
                        )
                        e = out_engines[n_out_dma % len(out_engines)]
                        n_out_dma += 1
                        eng(e).dma_start(dst, src)

    nc.compile()
    return nc


def build_nc_v3(
    n_img: int,
    in_engine="s",
    out_engines="ssgc",
    u2_engines="vc",
    s0_engines="vc",
    utd_engine="v",
    t1d_engine="c",
    t2d_engines="cv",
    xst_bufs=6,
    u2sb_bufs=3,
    utsb_bufs=3,
    s0_bufs=2,
    s1_bufs=2,
    s2_bufs=2,
    ptp_bufs=4,
):
    """Channel-major output, f32r stage 1 (no input conversion).

    Per image (8 strips x 8 tiles of 128x128):
      S1 (PE, f32r x f32r, 1 cyc/row at 512-wide): per half-strip
         u2[16u+bi, c] = DT1.T @ xstrip            (PSUM f32)
      u2 drain -> u2sb fp16 [128, 1024]            (cast, contiguous)
      UT (PE): transpose 128-chunks -> UTsb[c_loc, tj*?? + 16u+bi] fp16
      S2 (PE, fp16): z2[16v+bj, (q,u,bi)] = DT1h.T @ UTsb_half
      S0[p0=16v+bj, ti*1024 + u*128 + tj*16 + bi]  (cast, 16-elem runs)
      T1 (PE): transpose S0[:, ti*1024+u*128 :+128] -> P1[tj*16+bi, 16v+bj]
      S1sb[p1, ti2*4096 + bj*256 + til*64 + u*8 + v]  (fp16, 8-elem runs)
      T2 (PE, half-width): S1sb 64-windows @ (ti2,bj,til)
         -> P2[ti2*64 + u*8 + v, tj*16+bi]
      S2sb[p2, til*2048 + bi*128 + tj*16 + bj]     (fp32, 4-elem runs)
      out DMA per (u, ti2): 8 partitions x 32KB contiguous descriptors.
    """
    import concourse.bacc as bacc
    import concourse.mybir as mybir
    import concourse.tile as tile

    f32 = mybir.dt.float32
    f32r = mybir.dt.float32r
    f16 = mybir.dt.float16
    nc = bacc.Bacc("TRN2", target_bir_lowering=False, debug=False)

    x = nc.dram_tensor("x", [n_img, 1, _H, _W], f32r, kind="ExternalInput")
    dt1 = nc.dram_tensor("dt1", [128, 128], f32r, kind="ExternalInput")
    dt1h = nc.dram_tensor("dt1h", [128, 128], f16, kind="ExternalInput")
    ident = nc.dram_tensor("ident", [128, 128], f16, kind="ExternalInput")
    out = nc.dram_tensor("out", [n_img, 64, 128, 128], f32, kind="ExternalOutput")

    def eng(ch):
        return {"s": nc.sync, "c": nc.scalar, "g": nc.gpsimd, "v": nc.vector}[ch]

    def ecopy(ch, out_ap, in_ap):
        if ch == "c":
            nc.scalar.copy(out_ap, in_ap)
        else:
            eng(ch).tensor_copy(out_ap, in_ap)

    cnt = {"u2": 0, "s0": 0, "t2": 0, "out": 0}

    with tile.TileContext(nc) as tc:
        with (
            tc.tile_pool(name="const", bufs=1) as constp,
            tc.tile_pool(name="xst", bufs=xst_bufs) as xsp,
            tc.tile_pool(name="u2sb", bufs=u2sb_bufs) as u2p,
            tc.tile_pool(name="utsb", bufs=utsb_bufs) as utp,
            tc.tile_pool(name="s0", bufs=s0_bufs) as s0p,
            tc.tile_pool(name="s1sb", bufs=s1_bufs) as s1p,
            tc.tile_pool(name="s2sb", bufs=s2_bufs) as s2p,
            tc.tile_pool(name="psu", bufs=2, space="PSUM") as psu,
            tc.tile_pool(name="psz", bufs=2, space="PSUM") as psz,
            tc.tile_pool(name="ptp", bufs=ptp_bufs, space="PSUM") as ptp,
        ):
            dt1_t = constp.tile([128, 128], f32r)
            nc.sync.dma_start(dt1_t[:], dt1[:])
            dt1h_t = constp.tile([128, 128], f16)
            nc.sync.dma_start(dt1h_t[:], dt1h[:])
            ident_t = constp.tile([128, 128], f16)
            nc.sync.dma_start(ident_t[:], ident[:])

            for img in range(n_img):
                s0 = s0p.tile([128, 8192], f16)
                for ti in range(8):
                    xst = xsp.tile([128, _W], f32r, tag="xstrip")
                    eng(in_engine).dma_start(
                        xst[:], x[img, 0, 128 * ti : 128 * (ti + 1), :]
                    )
                    # stage 1 (f32r): u2[16u+bi, c] per half-strip
                    u2sb = u2p.tile([128, 1024], f16)
                    for h in range(2):
                        u2_ps = psu.tile([128, 512], f32)
                        nc.tensor.matmul(
                            u2_ps[:],
                            dt1_t[:],
                            xst[:, h * 512 : (h + 1) * 512],
                            start=True,
                            stop=True,
                        )
                        e = u2_engines[cnt["u2"] % len(u2_engines)]
                        cnt["u2"] += 1
                        ecopy(e, u2sb[:, h * 512 : (h + 1) * 512], u2_ps[:])
                    # UT: back to [c_loc, 16u+bi] per 128-chunk
                    put = ptp.tile([128, 1024], f16, tag="tp")
                    for cc in range(8):
                        nc.tensor.transpose(
                            put[:, cc * 128 : (cc + 1) * 128],
                            u2sb[:, cc * 128 : (cc + 1) * 128],
                            ident_t[:],
                        )
                    utsb = utp.tile([128, 1024], f16)
                    ecopy(utd_engine, utsb[:], put[:])
                    # stage 2 (fp16) + S0 drain
                    for h in range(2):
                        z2 = psz.tile([128, 512], f32)
                        nc.tensor.matmul(
                            z2[:],
                            dt1h_t[:],
                            utsb[:, h * 512 : (h + 1) * 512],
                            start=True,
                            stop=True,
                        )
                        src = z2[:].rearrange("p (q u b) -> p q u b", q=4, u=8)
                        dst = s0[:].rearrange(
                            "p (t u j b) -> p t j u b", t=8, u=8, j=8
                        )[:, ti, h * 4 : h * 4 + 4, :, :]
                        e = s0_engines[cnt["s0"] % len(s0_engines)]
                        cnt["s0"] += 1
                        ecopy(e, dst, src)

                # ---- T1: per ti, 8 transposes (u) into one PSUM bank ----
                s1sb = s1p.tile([128, 8192], f16)
                s1v = s1sb[:].rearrange(
                    "p (w b t u v) -> p w t u v b", w=2, b=16, t=4, u=8
                )
                for ti in range(8):
                    ti2, til = ti // 4, ti % 4
                    pt = ptp.tile([128, 1024], f16, tag="tp")
                    for u in range(8):
                        nc.tensor.transpose(
                            pt[:, u * 128 : (u + 1) * 128],
                            s0[:, ti * 1024 + u * 128 : ti * 1024 + (u + 1) * 128],
                            ident_t[:],
                        )
                    src = pt[:].rearrange("p (u v b) -> p u v b", u=8, v=8)
                    dst = s1v[:, ti2, til, :, :, :]
                    ecopy(t1d_engine, dst, src)

                # ---- T2: per (til, g2), 16 half-width transposes ----
                s2sb = s2p.tile([128, 8192], f32)
                s2v = s2sb[:].rearrange("p (t b j c) -> p t c j b", t=4, b=16, j=8)
                for til in range(4):
                    for g2 in range(2):
                        pt2 = ptp.tile([128, 1024], f16, tag="tp")
                        for kk in range(8):
                            bj = g2 * 8 + kk
                            for ti2 in range(2):
                                nc.tensor.transpose(
                                    pt2[
                                        ti2 * 64 : ti2 * 64 + 64,
                                        kk * 128 : (kk + 1) * 128,
                                    ],
                                    s1sb[
                                        :,
                                        ti2 * 4096 + bj * 256 + til * 64 : ti2 * 4096
                                        + bj * 256
                                        + til * 64
                                        + 64,
                                    ],
                                    ident_t[:],
                                )
                        src = pt2[:].rearrange("p (c t b) -> p c t b", c=8, t=8)
                        dst = s2v[:, til, g2 * 8 : g2 * 8 + 8, :, :]
                        e = t2d_engines[cnt["t2"] % len(t2d_engines)]
                        cnt["t2"] += 1
                        ecopy(e, dst, src)

                # ---- output: per (u, ti2): 8 partitions x 32KB ----
                for u in range(8):
                    for ti2 in range(2):
                        src = s2sb[ti2 * 64 + u * 8 : ti2 * 64 + (u + 1) * 8, :]
                        dst = out[
                            img, 8 * u : 8 * u + 8, ti2 * 64 : ti2 * 64 + 64, :
                        ].rearrange("v r j -> v (r j)")
                        e = out_engines[cnt["out"] % len(out_engines)]
                        cnt["out"] += 1
                        eng(e).dma_start(dst, src)

    nc.compile()
    return nc


def _get_nc(n_img: int):
    if n_img not in _NC_CACHE:
        _NC_CACHE[n_img] = build_nc(n_img, strip_input=STRIP_INPUT)
    return _NC_CACHE[n_img]


def _split_f16(m: np.ndarray):
    hi = m.astype(np.float16)
    lo = (m - hi.astype(np.float32)).astype(np.float16)
    return hi, lo


def make_inputs(x_core: np.ndarray, dct: np.ndarray, host_split=False, v2=False) -> dict:
    dt1 = _build_dt1(dct)
    dt1h, dt1l = _split_f16(dt1)
    if v2:
        ident = np.eye(128, dtype=np.float16)
        return {"x": x_core, "dt1": dt1, "dt1h": dt1h, "ident": ident}
    if host_split:
        xh = x_core.astype(np.float16)
        xl = (x_core - xh.astype(np.float32)).astype(np.float16)
        x_core = np.concatenate((xh, xl), axis=-1)
    return {"x": x_core, "dt1": dt1, "dt1h": dt1h, "dt1l": dt1l}


def run_spmd(
    x: np.ndarray, dct: np.ndarray, trace: bool = False, nc=None, host_split=HOST_SPLIT
):
    """Run the SPMD kernel on 8 cores. Returns (out, BassKernelResults)."""
    from concourse.bass_utils import run_bass_kernel_spmd

    x = np.ascontiguousarray(np.asarray(x, dtype=np.float32))
    dct = np.asarray(dct, dtype=np.float32)
    b = x.shape[0]
    per = b // _N_CORES

    if nc is None:
        nc = _get_nc(per)
    in_maps = [
        make_inputs(x[i * per : (i + 1) * per], dct, host_split=host_split)
        for i in range(_N_CORES)
    ]
    res = run_bass_kernel_spmd(
        nc, in_maps, core_ids=list(range(_N_CORES)), trace=trace
    )
    out = np.concatenate(
        [res.results[i]["out"] for i in range(_N_CORES)], axis=0
    )
    return out, res


def kernel(x, dct=None):
    if dct is None:
        dct = _dct_mat_np()
    out, _ = run_spmd(x, dct, trace=False)
    return out

